# revision 9
# baseline (speedup 1.0000x reference)
"""RLeaky SNN scan kernel for Trainium2 (8 NeuronCores, batch data-parallel).

Per core: B_local=32 batch rows, full T=128 scan.

Layout: everything lives in "f-major packed" [128, 512] tiles:
    tile[p, 32*fb + i]  =  logical element (batch row i, feature f = 128*fb + p)
(p = partition, fb = 0..15 feature block, i = 0..31 batch row).

This is exactly the transposed orientation the XLA reference uses for the
recurrent dot, so the matmul consumes spk^T and produces dot^T with NO
per-step transposes, full M=128 array utilization, and — critically — the
same fp32 LOW/HIGH-pass accumulation order as the reference (W stationary),
which makes the whole scan bitwise-reproducible against it.

Per step t:
  dot^T[fb] = sum_jb  WT_block(jb, fb).T @ spkT_block(jb)   (PSUM, jb ascending)
  u1   = 0.95 * mem      (ACT)
  u2   = u1 + x_t        (DVE)
  u3   = u2 + dot        (DVE, reads PSUM)
  u4   = u3 + b          (DVE)
  mem' = u4 - spk        (DVE; reset == spk bitwise)
  spk' = (mem' > 1.0)    (DVE is_gt)
"""

import sys

if "/opt/trn_rl_repo" not in sys.path:
    sys.path.insert(0, "/opt/trn_rl_repo")

import numpy as np

import concourse.mybir as mybir
import concourse.tile as tile
from concourse import bacc
from concourse.bass_utils import run_bass_kernel_spmd

F32 = mybir.dt.float32

B, T_FULL, F = 256, 128, 2048
NCORES = 8
BL = B // NCORES  # 32 batch rows per core

_nc_cache = {}


def _build(T=T_FULL, repeat=1):
    key = (T, repeat)
    if key in _nc_cache:
        return _nc_cache[key]

    nc = bacc.Bacc(None, target_bir_lowering=False)
    xp_d = nc.dram_tensor("xp", [T, 128, 512], F32, kind="ExternalInput")
    wt_d = nc.dram_tensor("wt", [F, F], F32, kind="ExternalInput")  # W.T, [j, f]
    bp_d = nc.dram_tensor("bp", [128, 512], F32, kind="ExternalInput")
    spk_out = nc.dram_tensor("spk_out", [T, 128, 512], F32, kind="ExternalOutput")
    mem_out = nc.dram_tensor("mem_out", [T, 128, 512], F32, kind="ExternalOutput")

    with tile.TileContext(nc) as tc:
        with (
            tc.tile_pool(name="wpool", bufs=1) as wpool,
            tc.tile_pool(name="wdma", bufs=2) as wdma,
            tc.tile_pool(name="const", bufs=1) as const,
            tc.tile_pool(name="state", bufs=2) as state,
            tc.tile_pool(name="xtp", bufs=3) as xtp,
            tc.tile_pool(name="tmp", bufs=2) as tmp,
            tc.tile_pool(name="pmm", bufs=2, space="PSUM") as pmm,
        ):
            # --- init: stage W.T into SBUF via DVE (PE may only read DVE-written
            # tiles so every PE instruction needs at most one sync wait)
            wt_sb = wpool.tile([128, 16 * F], F32)
            for jb in range(16):
                wchunk = wdma.tile([128, F], F32, tag="wchunk")
                nc.gpsimd.dma_start(wchunk[:], wt_d[jb * 128 : (jb + 1) * 128, :])
                nc.vector.tensor_copy(wt_sb[:, jb * F : (jb + 1) * F], wchunk[:])

            bp_sb = const.tile([128, 512], F32)
            nc.gpsimd.dma_start(bp_sb[:], bp_d[:])

            mem_cur = state.tile([128, 512], F32, tag="mem", name="mem0")
            nc.vector.memset(mem_cur[:], 0.0)
            spk_cur = state.tile([128, 512], F32, tag="spk", name="spk0")
            nc.vector.memset(spk_cur[:], 0.0)

            def scan_body(mem_in, spk_in, rep="r"):
              mem_cur, spk_cur = mem_in, spk_in
              for t in range(T):
                xt = xtp.tile([128, 512], F32, tag="xt", name=f"xt{rep}_{t}")
                nc.gpsimd.dma_start(xt[:], xp_d[t, :, :])

                # --- dot^T: for each output f-block, accumulate over j-blocks
                # ascending; W.T block is the stationary operand (its fp32
                # LOW/HIGH split accumulation matches the XLA reference).
                mm_ps = pmm.tile([128, 512], F32, tag="mm", name=f"mm{t}")
                for fb in range(16):
                    for jb in range(16):
                        nc.tensor.matmul(
                            mm_ps[:, 32 * fb : 32 * (fb + 1)],
                            wt_sb[:, jb * F + fb * 128 : jb * F + fb * 128 + 128],
                            spk_cur[:, 32 * jb : 32 * (jb + 1)],
                            start=(jb == 0),
                            stop=(jb == 15),
                        )

                # --- elementwise chain (association matches the reference)
                u1 = tmp.tile([128, 512], F32, tag="u1", name=f"u1_{t}")
                nc.scalar.mul(u1[:], mem_cur[:], 0.95)
                u2 = tmp.tile([128, 512], F32, tag="u2", name=f"u2_{t}")
                nc.vector.tensor_add(u2[:], u1[:], xt[:])
                u3 = tmp.tile([128, 512], F32, tag="u3", name=f"u3_{t}")
                nc.vector.tensor_add(u3[:], u2[:], mm_ps[:])
                u4 = tmp.tile([128, 512], F32, tag="u4", name=f"u4_{t}")
                nc.vector.tensor_add(u4[:], u3[:], bp_sb[:])
                mem_new = state.tile([128, 512], F32, tag="mem", name=f"mem{t + 1}")
                nc.vector.tensor_sub(mem_new[:], u4[:], spk_cur[:])
                spk_new = state.tile([128, 512], F32, tag="spk", name=f"spk{t + 1}")
                nc.vector.tensor_scalar(
                    spk_new[:], mem_new[:], 1.0, None, mybir.AluOpType.is_gt
                )

                nc.gpsimd.dma_start(mem_out[t, :, :], mem_new[:])
                nc.gpsimd.dma_start(spk_out[t, :, :], spk_new[:])

                mem_cur = mem_new
                spk_cur = spk_new
              return mem_cur, spk_cur

            if repeat == 1:
                scan_body(mem_cur, spk_cur)
            else:
                # timing mode: run the scan `repeat` times inside one NEFF
                # (state tiles are re-read from the trace-time handles; data
                # is garbage after the first iteration, which is fine for
                # wall-clock benchmarking)
                with tc.For_i(0, repeat, 1):
                    scan_body(mem_cur, spk_cur)

    nc.compile()
    _nc_cache[key] = nc
    return nc


def _pack_x(xc, T):
    # [32, T, 2048] -> [T, 128, 512] f-major packed:
    # out[t, p, 32*fb + i] = xc[i, t, 128*fb + p]
    a = xc.transpose(1, 2, 0)  # [T, 2048, 32]
    a = a.reshape(T, 16, 128, 32).transpose(0, 2, 1, 3)  # [T, 128, 16, 32]
    return np.ascontiguousarray(a.reshape(T, 128, 512))


def _unpack_rec(a, T):
    # [T, 128, 512] f-major packed -> [32, T, 2048]
    a = a.reshape(T, 128, 16, 32).transpose(0, 2, 1, 3)  # [T, 16, 128, 32]
    a = a.reshape(T, 2048, 32)
    return np.ascontiguousarray(a.transpose(2, 0, 1))


def kernel(x, W, b, T=None, trace=False, repeat=1):
    x = np.asarray(x, dtype=np.float32)
    W = np.asarray(W, dtype=np.float32)
    b = np.asarray(b, dtype=np.float32)
    if T is None:
        T = x.shape[1]
    x = x[:, :T, :]

    nc = _build(T, repeat=repeat)
    Wt = np.ascontiguousarray(W.T)
    # bp[p, 32*fb + i] = b[128*fb + p]
    bp = np.ascontiguousarray(
        np.repeat(b.reshape(16, 128).T[:, :, None], 32, axis=2).reshape(128, 512)
    )

    in_maps = []
    for c in range(NCORES):
        xc = x[c * BL : (c + 1) * BL]  # [32, T, 2048]
        in_maps.append({"xp": _pack_x(xc, T), "wt": Wt, "bp": bp})

    try:
        res = run_bass_kernel_spmd(
            nc, in_maps, core_ids=list(range(NCORES)), trace=trace
        )
    except ModuleNotFoundError:
        # no axon NTFF profiling hook in this environment; run without trace
        res = run_bass_kernel_spmd(
            nc, in_maps, core_ids=list(range(NCORES)), trace=False
        )
    spk_parts = []
    mem_parts = []
    for c in range(NCORES):
        spk_parts.append(_unpack_rec(res.results[c]["spk_out"], T))
        mem_parts.append(_unpack_rec(res.results[c]["mem_out"], T))
    spk_rec = np.concatenate(spk_parts, axis=0)
    mem_rec = np.concatenate(mem_parts, axis=0)
    if trace:
        kernel.last_result = res
    return spk_rec, mem_rec

